# revision 11
# baseline (speedup 1.0000x reference)
"""Trainium2 Bass kernel for nn_CustomRNNCell (Kuramoto-style RNN cell).

Strategy: pure data parallelism over the batch dim (B=512 -> 64 rows/core,
8 cores), parameters replicated.  All activations live on-chip in a
"transposed" layout [feature, batch] so every weight matrix is consumed by
the PE untransposed; the host does the input transposes / output
un-transposes / tensor packing (pure data movement).

Key algebraic restructurings (validated against the reference):
  * delta_term = sin(d)*(F @ cos(d)) - cos(d)*(F @ sin(d))   (angle-difference
    expansion; kills the [B,A,A] sin grid)
  * piecewise-linear basis: with c = cumsum(b_t0^2) (knots, increasing),
      sum_i w_i*relu(f - c_i) = W63*relu(f) - sum_i w_i*min(c_i, relu(f))
    (the sum-w*c constants cancel between the two relu branches), computed
    with broadcast access patterns on the DVE; the fat min/mult/fold passes
    only ever see values <= c_max ~ 0.6 so they run in bf16, while the
    dominant W63*relu(f) term stays fp32.
  * clip(x,-m,m) = min(max(x,-m),m) as one tensor_scalar op.
  * new_state via one PSUM-accumulated matmul over the stacked
    [1; prev; inputs; delta_term; action] x [st3; st1; st3_Pm; st2; st4];
    everything not needing `action` accumulates while the basis runs.
  * params are host-packed into a handful of [128, N] panels -> one DMA each
    (the HWDGE ring serializes DMAs at ~0.6us apiece).
"""

import sys

for _p in ("/opt/trn_rl_repo",):
    if _p not in sys.path:
        sys.path.insert(0, _p)

import numpy as np

import concourse.bacc as bacc
import concourse.mybir as mybir
import concourse.tile as tile
from concourse.bass_utils import run_bass_kernel_spmd
from concourse.masks import make_identity

B, A, I = 512, 256, 64
S, P = 512, 256
NCORES = 8
BL = B // NCORES  # 64 batch rows per core

DT = mybir.dt.float32
BF = mybir.dt.bfloat16
AX = mybir.AxisListType
ALU = mybir.AluOpType
ACTF = mybir.ActivationFunctionType

PI = float(np.pi)


def build_nc():
    nc = bacc.Bacc()

    # ---- DRAM I/O (host-packed panels) -------------------------------
    d_bp4 = nc.dram_tensor("bp4", [I, 4 * A], DT, kind="ExternalInput")
    d_rec2 = nc.dram_tensor("rec2", [I, 2 * I], DT, kind="ExternalInput")
    d_prevp = nc.dram_tensor("prevp", [128, 4 * BL], DT, kind="ExternalInput")
    d_selcat = nc.dram_tensor("selcatp", [128, 4 * 2 * A], DT, kind="ExternalInput")
    d_sawma = nc.dram_tensor("sawma", [128, 5], DT, kind="ExternalInput")
    d_stFTp = nc.dram_tensor("stFTp", [128, 2 * A], DT, kind="ExternalInput")
    d_inpp = nc.dram_tensor("inpp", [128, 2 * BL], DT, kind="ExternalInput")
    d_st3 = nc.dram_tensor("st3", [1, S], DT, kind="ExternalInput")
    d_wnsp = nc.dram_tensor("wnsp", [128, 10 * S], DT, kind="ExternalInput")

    d_ns = nc.dram_tensor("ns_out", [BL, S], DT, kind="ExternalOutput")
    d_freq = nc.dram_tensor("freq_out", [BL, A], DT, kind="ExternalOutput")
    d_loss = nc.dram_tensor("loss_out", [BL, 1], DT, kind="ExternalOutput")
    d_actp = nc.dram_tensor("actp_out", [128, 2 * BL], DT, kind="ExternalOutput")

    with tile.TileContext(nc) as tc:
        with (
            tc.tile_pool(name="const", bufs=1) as cpool,
            tc.tile_pool(name="work", bufs=2) as wpool,
            tc.tile_pool(name="fat", bufs=2) as fatpool,
            tc.tile_pool(name="psum", bufs=6, space="PSUM") as ppool,
            tc.tile_pool(name="psum_ns", bufs=1, space="PSUM") as ppool_ns,
        ):
            # ---- input DMAs: one per panel, in order of need -----------
            bp4 = cpool.tile([I, 4 * A], DT, tag="bp4")
            nc.sync.dma_start(out=bp4[:], in_=d_bp4[:])
            rec2 = cpool.tile([I, 2 * I], DT, tag="rec2")
            nc.sync.dma_start(out=rec2[:], in_=d_rec2[:])
            prevp = cpool.tile([128, 4 * BL], DT, tag="prevp")
            nc.sync.dma_start(out=prevp[:], in_=d_prevp[:])
            selcat = cpool.tile([128, 4 * 2 * A], DT, tag="selcat")
            nc.sync.dma_start(out=selcat[:], in_=d_selcat[:])
            sawma = cpool.tile([128, 5], DT, tag="sawma")
            nc.sync.dma_start(out=sawma[:], in_=d_sawma[:])
            # the late-needed big/bulky loads go on the ACT HWDGE ring
            stFTp = cpool.tile([128, 2 * A], DT, tag="stFTp")
            nc.scalar.dma_start(out=stFTp[:], in_=d_stFTp[:])
            inpp = cpool.tile([128, 2 * BL], DT, tag="inpp")
            nc.scalar.dma_start(out=inpp[:], in_=d_inpp[:])
            st3 = cpool.tile([1, S], DT, tag="st3")
            nc.scalar.dma_start(out=st3[:], in_=d_st3[:])
            wnsp = cpool.tile([128, 10 * S], DT, tag="wnsp")
            nc.scalar.dma_start(out=wnsp[:], in_=d_wnsp[:])

            def prevT(k):
                return prevp[:, k * BL:(k + 1) * BL]

            def selc(k):  # [128, 512] K-tile of [select_w | select_delta]
                return selcat[:, k * 2 * A:(k + 1) * 2 * A]

            def wns(k):
                return wnsp[:, k * S:(k + 1) * S]

            ident = cpool.tile([BL, BL], DT, tag="ident")
            make_identity(nc, ident[:])
            ones_row = cpool.tile([1, BL], DT, tag="ones_row")
            nc.vector.memset(ones_row[:], 1.0)
            ma_col = sawma[:, 4:5]
            nma_col = cpool.tile([128, 1], DT, tag="nma")
            nc.vector.tensor_scalar(nma_col[:], ma_col, -1.0, None, ALU.mult)
            bias_hpi = cpool.tile([128, 1], DT, tag="bias_hpi")
            nc.vector.memset(bias_hpi[:], PI / 2)

            # ---- param prep: squares (DVE; avoids the ACT Square-table
            # load) + small matmuls ---------------------------------------
            sq = {}
            for j, nm in enumerate(("wp", "bp", "wm", "bm")):
                t = wpool.tile([I, A], DT, tag=f"sq_{nm}")
                src = bp4[:, j * A:(j + 1) * A]
                nc.vector.tensor_tensor(t[:], src, src, ALU.mult)
                sq[nm] = t
            wrec = rec2[:, 0:I]
            brec = rec2[:, I:2 * I]

            # w_plus = wp2 @ wrec ; w_minus = -(wm2 @ wrec)
            # c = bp2 @ brec ; c' = bm2 @ brec   (knots; = -b_plus / -b_minus)
            # w in bf16 for the fat passes; W63 = sum_i w_i in fp32 from PSUM.
            wpb, wmb, cK, cKm, W63p, W63m = [], [], [], [], [], []
            for half in range(2):
                ms = slice(half * 128, (half + 1) * 128)
                for nm, rhs, neg in (("wp", wrec, False), ("wm", wrec, True),
                                     ("bp", brec, False), ("bm", brec, False)):
                    ps = ppool.tile([128, I], DT, tag="ps")
                    nc.tensor.matmul(ps[:], sq[nm][:, ms], rhs, start=True, stop=True)
                    if nm in ("wp", "wm"):
                        t = wpool.tile([128, I], BF, tag=f"{nm}_{half}")
                        nc.scalar.activation(t[:], ps[:], ACTF.Copy,
                                             scale=-1.0 if neg else 1.0)
                        (wpb if nm == "wp" else wmb).append(t)
                        w63 = wpool.tile([128, 1], DT, tag=f"w63_{nm}{half}")
                        nc.vector.tensor_reduce(w63[:], ps[:], AX.X, ALU.add,
                                                negate=neg)
                        (W63p if nm == "wp" else W63m).append(w63)
                    else:
                        t = wpool.tile([128, I], DT, tag=f"{nm}_{half}")
                        nc.scalar.activation(t[:], ps[:], ACTF.Copy)
                        (cK if nm == "bp" else cKm).append(t)

            # ---- freq / delta:  fdT = sel_cat^T-as-lhsT @ prevT ----------
            fd_ps = []
            for m in range(4):
                ps = ppool.tile([128, BL], DT, tag="ps")
                for k in range(4):
                    nc.tensor.matmul(ps[:], selc(k)[:, m * 128:(m + 1) * 128],
                                     prevT(k), start=(k == 0), stop=(k == 3))
                fd_ps.append(ps)

            # r+ = relu(f), r- = relu(-f)  (fp32, straight from PSUM)
            r_p, r_m = [], []
            for half in range(2):
                rp = wpool.tile([128, BL], DT, tag=f"r_p{half}")
                nc.vector.tensor_scalar(rp[:], fd_ps[half][:], 0.0, None, ALU.max)
                rm = wpool.tile([128, BL], DT, tag=f"r_m{half}")
                nc.vector.tensor_scalar(rm[:], fd_ps[half][:], -1.0, 0.0,
                                        ALU.mult, ALU.max)
                r_p.append(rp)
                r_m.append(rm)

            # ---- sin / cos of delta (range-reduced) ----------------------
            # y = x - 2pi*k via an int32 cast (round-to-nearest on HW,
            # trunc in CoreSim); a branch-free +-2pi correction makes the
            # result [-pi, pi] under either conversion mode.
            sinT, cosT = [], []
            for half in range(2):
                ki = wpool.tile([128, BL], mybir.dt.int32, tag="sc_ki")
                nc.vector.tensor_scalar(ki[:], fd_ps[2 + half][:],
                                        float(1 / (2 * PI)), 32.0, ALU.mult, ALU.add)
                xoff = wpool.tile([128, BL], DT, tag="sc_xoff")
                nc.vector.tensor_scalar(xoff[:], fd_ps[2 + half][:], float(64 * PI),
                                        None, ALU.add)
                y1 = wpool.tile([128, BL], DT, tag="sc_y1")
                nc.vector.scalar_tensor_tensor(y1[:], ki[:], float(-2 * PI), xoff[:],
                                               ALU.mult, ALU.add)
                w = wpool.tile([128, BL], DT, tag="sc_w")
                nc.vector.tensor_scalar(w[:], y1[:], PI, float(-2 * PI),
                                        ALU.is_gt, ALU.mult)
                y2 = wpool.tile([128, BL], DT, tag="sc_y2")
                nc.vector.tensor_tensor(y2[:], y1[:], w[:], ALU.add)
                y = wpool.tile([128, BL], DT, tag="sc_y")
                nc.vector.tensor_scalar(y[:], y2[:], -PI, PI, ALU.max, ALU.min)
                s = wpool.tile([128, BL], DT, tag=f"sinT{half}")
                nc.scalar.activation(s[:], y[:], ACTF.Sin)
                # cos(y) = sin(pi/2 - |y|),  argument stays in [-pi/2, pi/2]
                ay = wpool.tile([128, BL], DT, tag="sc_ay")
                nc.scalar.activation(ay[:], y[:], ACTF.Abs)
                c = wpool.tile([128, BL], DT, tag=f"cosT{half}")
                nc.scalar.activation(c[:], ay[:], ACTF.Sin, bias=bias_hpi[:],
                                     scale=-1.0)
                sinT.append(s)
                cosT.append(c)

            # ---- U = F @ cos, V = F @ sin ; dtT = sin*U - cos*V ----------
            dtT = []
            for m in range(2):
                psU = ppool.tile([128, BL], DT, tag="ps")
                psV = ppool.tile([128, BL], DT, tag="ps")
                for k in range(2):
                    lhs = stFTp[:, k * A + m * 128:k * A + (m + 1) * 128]
                    nc.tensor.matmul(psU[:], lhs, cosT[k][:], start=(k == 0), stop=(k == 1))
                    nc.tensor.matmul(psV[:], lhs, sinT[k][:], start=(k == 0), stop=(k == 1))
                t1 = wpool.tile([128, BL], DT, tag="dt_t1")
                nc.vector.tensor_tensor(t1[:], sinT[m][:], psU[:], ALU.mult)
                t2 = wpool.tile([128, BL], DT, tag="dt_t2")
                nc.vector.tensor_tensor(t2[:], cosT[m][:], psV[:], ALU.mult)
                t = wpool.tile([128, BL], DT, tag=f"dtT{m}")
                nc.vector.tensor_tensor(t[:], t1[:], t2[:], ALU.subtract)
                dtT.append(t)

            # ---- new_state stacked matmul: everything that doesn't need
            # action accumulates into PSUM while the basis runs -------------
            ns_ps = ppool_ns.tile([BL, S], DT, tag="ns_ps")
            nc.tensor.matmul(ns_ps[:], ones_row[:], st3[:], start=True, stop=False)
            for k in range(4):
                nc.tensor.matmul(ns_ps[:], prevT(k), wns(k), start=False, stop=False)
            for k in range(2):
                nc.tensor.matmul(ns_ps[:], inpp[:, k * BL:(k + 1) * BL], wns(6 + k),
                                 start=False, stop=False)
            for k in range(2):
                nc.tensor.matmul(ns_ps[:], dtT[k][:], wns(4 + k),
                                 start=False, stop=False)

            # ---- piecewise-linear basis (the fat part) -------------------
            # anc = W63p*r+ - sum_i wp_i*min(c_i,r+)
            #     + W63m*r- - sum_i wm_i*min(c'_i,r-)
            actp = wpool.tile([128, 2 * BL], DT, tag="actp")
            actT = []
            for half in range(2):
                tbr = []
                for (r, wb, cc, w63) in (
                    (r_p[half], wpb[half], cK[half], W63p[half]),
                    (r_m[half], wmb[half], cKm[half], W63m[half]),
                ):
                    r_bc = r[:].rearrange("p (b u) -> p b u", u=1).broadcast_to([128, BL, I])
                    c_bc = cc[:].rearrange("p (u i) -> p u i", u=1).broadcast_to([128, BL, I])
                    w_bc = wb[:].rearrange("p (u i) -> p u i", u=1).broadcast_to([128, BL, I])
                    M = fatpool.tile([128, BL * I], BF, tag="M")
                    M3 = M[:].rearrange("p (b i) -> p b i", i=I)
                    nc.vector.tensor_tensor(M3, r_bc, c_bc, ALU.min)
                    Y = fatpool.tile([128, BL * I], BF, tag="Y")
                    Y3 = Y[:].rearrange("p (b i) -> p b i", i=I)
                    nc.vector.tensor_tensor(Y3, M3, w_bc, ALU.mult)
                    F1 = fatpool.tile([128, BL * 32], BF, tag="F1")
                    F13 = F1[:].rearrange("p (b i) -> p b i", i=32)
                    nc.vector.tensor_tensor(F13, Y3[:, :, 0:32], Y3[:, :, 32:64], ALU.add)
                    F2 = fatpool.tile([128, BL * 16], BF, tag="F2")
                    F23 = F2[:].rearrange("p (b i) -> p b i", i=16)
                    nc.vector.tensor_tensor(F23, F13[:, :, 0:16], F13[:, :, 16:32], ALU.add)
                    red = wpool.tile([128, BL], DT, tag="red")
                    nc.vector.tensor_reduce(red[:], F23, AX.X, ALU.add)
                    t = wpool.tile([128, BL], DT, tag="tbr")
                    nc.vector.scalar_tensor_tensor(t[:], r[:], w63[:], red[:],
                                                   ALU.mult, ALU.subtract)
                    tbr.append(t)
                anc = wpool.tile([128, BL], DT, tag="anc")
                nc.vector.tensor_tensor(anc[:], tbr[0][:], tbr[1][:], ALU.add)
                at = actp[:, half * BL:(half + 1) * BL]
                nc.vector.tensor_scalar(at, anc[:], nma_col[:], ma_col,
                                        ALU.max, ALU.min)
                actT.append(at)
            nc.sync.dma_start(out=d_actp[:], in_=actp[:])

            # action rows of the stacked matmul (wns blocks 8,9 = st4)
            for k in range(2):
                nc.tensor.matmul(ns_ps[:], actT[k], wns(8 + k),
                                 start=False, stop=(k == 1))

            ns_nat = wpool.tile([BL, S], DT, tag="ns_nat")
            nc.scalar.activation(ns_nat[:], ns_ps[:], ACTF.Copy)
            nc.sync.dma_start(out=d_ns[:], in_=ns_nat[:])

            # ---- transpose new_state back to [s, b] for the out matmuls --
            nsT, ns2T = [], []
            for k in range(4):
                ps = ppool.tile([128, BL], DT, tag="ps")
                nc.tensor.transpose(ps[:], ns_nat[:, k * 128:(k + 1) * 128], ident[:])
                t = wpool.tile([128, BL], DT, tag=f"nsT{k}")
                nc.scalar.activation(t[:], ps[:], ACTF.Copy)
                nsT.append(t)
                t2 = wpool.tile([128, BL], DT, tag=f"ns2T{k}")
                nc.vector.tensor_tensor(t2[:], t[:], t[:], ALU.mult)
                ns2T.append(t2)

            # ---- frequency = ns @ select_w ; loss0 = ns^2 @ saw ----------
            fq_ps = ppool.tile([BL, A], DT, tag="ps")
            for k in range(4):
                nc.tensor.matmul(fq_ps[:], nsT[k][:], selc(k)[:, 0:A],
                                 start=(k == 0), stop=(k == 3))
            fq = wpool.tile([BL, A], DT, tag="fq")
            nc.scalar.activation(fq[:], fq_ps[:], ACTF.Copy)
            nc.scalar.dma_start(out=d_freq[:], in_=fq[:])

            ls_ps = ppool.tile([BL, 1], DT, tag="ps")
            for k in range(4):
                nc.tensor.matmul(ls_ps[:], ns2T[k][:], sawma[:, k:k + 1],
                                 start=(k == 0), stop=(k == 3))
            ls = wpool.tile([BL, 1], DT, tag="ls")
            nc.scalar.activation(ls[:], ls_ps[:], ACTF.Copy)
            nc.scalar.dma_start(out=d_loss[:], in_=ls[:])

    nc.compile()
    return nc


_NC_CACHE = None


def _get_nc():
    global _NC_CACHE
    if _NC_CACHE is None:
        _NC_CACHE = build_nc()
    return _NC_CACHE


def _pack_rows(a, width):
    """[R, C] with R = 128*n  ->  [128, n*C] panel (blocks along free dim)."""
    r, c = a.shape
    n = r // 128
    return np.ascontiguousarray(
        a.reshape(n, 128, c).transpose(1, 0, 2).reshape(128, n * c))


def prepare_in_maps(inputs):
    x = {k: np.asarray(v) for k, v in inputs.items()}
    f32 = lambda a: np.ascontiguousarray(a, dtype=np.float32)

    sel_cat = np.concatenate([x["select_w"], x["select_delta"]], axis=1)
    wns = np.concatenate([x["state_transfer1"], x["state_transfer2"],
                          x["state_transfer3_Pm"], x["state_transfer4"]], axis=0)
    sawma = np.zeros((128, 5), np.float32)
    sawma[:, 0:4] = x["select_add_w"].reshape(4, 128).T
    sawma[:, 4] = np.float32(x["max_action"])

    shared = {
        "bp4": f32(np.concatenate([x["w_plus_temp0"].T, x["b_plus_temp0"].T,
                                   x["w_minus_temp0"].T, x["b_minus_temp0"].T],
                                  axis=1)),
        "rec2": f32(np.concatenate([x["w_recover"], x["b_recover"]], axis=1)),
        "selcatp": f32(_pack_rows(sel_cat, 2 * A)),
        "sawma": sawma,
        "stFTp": f32(_pack_rows(x["state_transferF"].T, A)),
        "st3": f32(x["state_transfer3"][None, :]),
        "wnsp": f32(_pack_rows(wns, S)),
    }
    in_maps = []
    for k in range(NCORES):
        sl = slice(k * BL, (k + 1) * BL)
        m = dict(shared)
        m["prevp"] = f32(_pack_rows(x["prev_output"][sl].T, BL))
        m["inpp"] = f32(_pack_rows(x["inputs"][sl].T, BL))
        in_maps.append(m)
    return in_maps


def postprocess(res):
    loss0 = np.concatenate([res[k]["loss_out"] for k in range(NCORES)], axis=0)
    frequency = np.concatenate([res[k]["freq_out"] for k in range(NCORES)], axis=0)
    action = np.concatenate(
        [np.concatenate([res[k]["actp_out"][:, 0:BL],
                         res[k]["actp_out"][:, BL:2 * BL]], axis=0).T
         for k in range(NCORES)], axis=0)
    new_state = np.concatenate([res[k]["ns_out"] for k in range(NCORES)], axis=0)
    return (loss0, frequency, action, new_state)


def kernel(**inputs):
    in_maps = prepare_in_maps(inputs)
    res = run_bass_kernel_spmd(_get_nc(), in_maps, list(range(NCORES))).results
    return postprocess(res)


# revision 12
# speedup vs baseline: 1.1264x; 1.1264x over previous
"""Trainium2 Bass kernel for nn_CustomRNNCell (Kuramoto-style RNN cell).

Strategy: pure data parallelism over the batch dim (B=512 -> 64 rows/core,
8 cores), parameters replicated.  All activations live on-chip in a
"transposed" layout [feature, batch] so every weight matrix is consumed by
the PE untransposed; the host does the input transposes / output
un-transposes / tensor packing (pure data movement).

Key algebraic restructurings (validated against the reference):
  * delta_term = sin(d)*(F @ cos(d)) - cos(d)*(F @ sin(d))   (angle-difference
    expansion; kills the [B,A,A] sin grid)
  * piecewise-linear basis: with c = cumsum(b_t0^2) (knots, increasing),
      sum_i w_i*relu(f - c_i) = W63*relu(f) - sum_i w_i*min(c_i, relu(f))
    (the sum-w*c constants cancel between the two relu branches), computed
    with broadcast access patterns on the DVE; the fat min/mult/fold passes
    only ever see values <= c_max ~ 0.6 so they run in bf16, while the
    dominant W63*relu(f) term stays fp32.
  * clip(x,-m,m) = min(max(x,-m),m) as one tensor_scalar op.
  * new_state via one PSUM-accumulated matmul over the stacked
    [1; prev; inputs; delta_term; action] x [st3; st1; st3_Pm; st2; st4];
    everything not needing `action` accumulates while the basis runs.
  * params are host-packed into a handful of [128, N] panels -> one DMA each
    (the HWDGE ring serializes DMAs at ~0.6us apiece).
"""

import sys

for _p in ("/opt/trn_rl_repo",):
    if _p not in sys.path:
        sys.path.insert(0, _p)

import numpy as np

import concourse.bacc as bacc
import concourse.mybir as mybir
import concourse.tile as tile
from concourse.bass_utils import run_bass_kernel_spmd
from concourse.masks import make_identity

B, A, I = 512, 256, 64
S, P = 512, 256
NCORES = 8
BL = B // NCORES  # 64 batch rows per core

DT = mybir.dt.float32
BF = mybir.dt.bfloat16
AX = mybir.AxisListType
ALU = mybir.AluOpType
ACTF = mybir.ActivationFunctionType

PI = float(np.pi)


def build_nc():
    nc = bacc.Bacc()

    # ---- DRAM I/O (host-packed panels) -------------------------------
    d_bp4 = nc.dram_tensor("bp4", [I, 4 * A], DT, kind="ExternalInput")
    d_rec2 = nc.dram_tensor("rec2", [I, 2 * I], DT, kind="ExternalInput")
    d_prevp = nc.dram_tensor("prevp", [128, 4 * BL], DT, kind="ExternalInput")
    d_selcat = nc.dram_tensor("selcatp", [128, 4 * 2 * A], DT, kind="ExternalInput")
    d_sawma = nc.dram_tensor("sawma", [128, 5], DT, kind="ExternalInput")
    d_stFTp = nc.dram_tensor("stFTp", [128, 2 * A], DT, kind="ExternalInput")
    d_inpp = nc.dram_tensor("inpp", [128, 2 * BL], DT, kind="ExternalInput")
    d_st3 = nc.dram_tensor("st3", [1, S], DT, kind="ExternalInput")
    d_wnsa = nc.dram_tensor("wnsa", [128, 5 * S], DT, kind="ExternalInput")
    d_wnsb = nc.dram_tensor("wnsb", [128, 5 * S], DT, kind="ExternalInput")

    d_ns = nc.dram_tensor("ns_out", [BL, S], DT, kind="ExternalOutput")
    d_freq = nc.dram_tensor("freq_out", [BL, A], DT, kind="ExternalOutput")
    d_loss = nc.dram_tensor("loss_out", [BL, 1], DT, kind="ExternalOutput")
    d_actp = nc.dram_tensor("actp_out", [128, 2 * BL], DT, kind="ExternalOutput")

    with tile.TileContext(nc) as tc:
        with (
            tc.tile_pool(name="const", bufs=1) as cpool,
            tc.tile_pool(name="work", bufs=2) as wpool,
            tc.tile_pool(name="fat", bufs=2) as fatpool,
            tc.tile_pool(name="psum", bufs=6, space="PSUM") as ppool,
            tc.tile_pool(name="psum_ns", bufs=1, space="PSUM") as ppool_ns,
        ):
            # ---- input DMAs: one per panel, in order of need -----------
            bp4 = cpool.tile([I, 4 * A], DT, tag="bp4")
            nc.sync.dma_start(out=bp4[:], in_=d_bp4[:])
            rec2 = cpool.tile([I, 2 * I], DT, tag="rec2")
            nc.sync.dma_start(out=rec2[:], in_=d_rec2[:])
            prevp = cpool.tile([128, 4 * BL], DT, tag="prevp")
            nc.sync.dma_start(out=prevp[:], in_=d_prevp[:])
            selcat = cpool.tile([128, 4 * 2 * A], DT, tag="selcat")
            nc.sync.dma_start(out=selcat[:], in_=d_selcat[:])
            wnsa = cpool.tile([128, 5 * S], DT, tag="wnsa")
            nc.sync.dma_start(out=wnsa[:], in_=d_wnsa[:])
            # the rest goes on the ACT HWDGE ring, in parallel
            sawma = cpool.tile([128, 5], DT, tag="sawma")
            nc.scalar.dma_start(out=sawma[:], in_=d_sawma[:])
            stFTp = cpool.tile([128, 2 * A], DT, tag="stFTp")
            nc.scalar.dma_start(out=stFTp[:], in_=d_stFTp[:])
            inpp = cpool.tile([128, 2 * BL], DT, tag="inpp")
            nc.scalar.dma_start(out=inpp[:], in_=d_inpp[:])
            st3 = cpool.tile([1, S], DT, tag="st3")
            nc.scalar.dma_start(out=st3[:], in_=d_st3[:])
            wnsb = cpool.tile([128, 5 * S], DT, tag="wnsb")
            nc.scalar.dma_start(out=wnsb[:], in_=d_wnsb[:])

            def prevT(k):
                return prevp[:, k * BL:(k + 1) * BL]

            def selc(k):  # [128, 512] K-tile of [select_w | select_delta]
                return selcat[:, k * 2 * A:(k + 1) * 2 * A]

            def wns(k):
                if k < 5:
                    return wnsa[:, k * S:(k + 1) * S]
                return wnsb[:, (k - 5) * S:(k - 5 + 1) * S]

            ident = cpool.tile([BL, BL], DT, tag="ident")
            make_identity(nc, ident[:])
            ones_row = cpool.tile([1, BL], DT, tag="ones_row")
            nc.vector.memset(ones_row[:], 1.0)
            ma_col = sawma[:, 4:5]
            nma_col = cpool.tile([128, 1], DT, tag="nma")
            nc.vector.tensor_scalar(nma_col[:], ma_col, -1.0, None, ALU.mult)
            bias_hpi = cpool.tile([128, 1], DT, tag="bias_hpi")
            nc.vector.memset(bias_hpi[:], PI / 2)

            # ---- param prep: squares (DVE; avoids the ACT Square-table
            # load) + small matmuls ---------------------------------------
            sq = {}
            for j, nm in enumerate(("wp", "bp", "wm", "bm")):
                t = wpool.tile([I, A], DT, tag=f"sq_{nm}")
                src = bp4[:, j * A:(j + 1) * A]
                nc.vector.tensor_tensor(t[:], src, src, ALU.mult)
                sq[nm] = t
            wrec = rec2[:, 0:I]
            brec = rec2[:, I:2 * I]

            # w_plus = wp2 @ wrec ; w_minus = -(wm2 @ wrec)
            # c = bp2 @ brec ; c' = bm2 @ brec   (knots; = -b_plus / -b_minus)
            # w in bf16 for the fat passes; W63 = sum_i w_i in fp32 from PSUM.
            wpb, wmb, cK, cKm, W63p, W63m = [], [], [], [], [], []
            for half in range(2):
                ms = slice(half * 128, (half + 1) * 128)
                for nm, rhs, neg in (("wp", wrec, False), ("wm", wrec, True),
                                     ("bp", brec, False), ("bm", brec, False)):
                    ps = ppool.tile([128, I], DT, tag="ps")
                    nc.tensor.matmul(ps[:], sq[nm][:, ms], rhs, start=True, stop=True)
                    if nm in ("wp", "wm"):
                        t = wpool.tile([128, I], BF, tag=f"{nm}_{half}")
                        nc.scalar.activation(t[:], ps[:], ACTF.Copy,
                                             scale=-1.0 if neg else 1.0)
                        (wpb if nm == "wp" else wmb).append(t)
                        w63 = wpool.tile([128, 1], DT, tag=f"w63_{nm}{half}")
                        nc.vector.tensor_reduce(w63[:], ps[:], AX.X, ALU.add,
                                                negate=neg)
                        (W63p if nm == "wp" else W63m).append(w63)
                    else:
                        t = wpool.tile([128, I], DT, tag=f"{nm}_{half}")
                        nc.scalar.activation(t[:], ps[:], ACTF.Copy)
                        (cK if nm == "bp" else cKm).append(t)

            # ---- freq / delta:  fdT = sel_cat^T-as-lhsT @ prevT ----------
            fd_ps = []
            for m in range(4):
                ps = ppool.tile([128, BL], DT, tag="ps")
                for k in range(4):
                    nc.tensor.matmul(ps[:], selc(k)[:, m * 128:(m + 1) * 128],
                                     prevT(k), start=(k == 0), stop=(k == 3))
                fd_ps.append(ps)

            # r+ = relu(f), r- = relu(-f)  (fp32, straight from PSUM)
            r_p, r_m = [], []
            for half in range(2):
                rp = wpool.tile([128, BL], DT, tag=f"r_p{half}")
                nc.vector.tensor_scalar(rp[:], fd_ps[half][:], 0.0, None, ALU.max)
                rm = wpool.tile([128, BL], DT, tag=f"r_m{half}")
                nc.vector.tensor_scalar(rm[:], fd_ps[half][:], -1.0, 0.0,
                                        ALU.mult, ALU.max)
                r_p.append(rp)
                r_m.append(rm)

            # ---- sin / cos of delta (range-reduced) ----------------------
            # y = x - 2pi*k via an int32 cast (round-to-nearest on HW,
            # trunc in CoreSim); a branch-free +-2pi correction makes the
            # result [-pi, pi] under either conversion mode.
            sinT, cosT = [], []
            for half in range(2):
                ki = wpool.tile([128, BL], mybir.dt.int32, tag="sc_ki")
                nc.vector.tensor_scalar(ki[:], fd_ps[2 + half][:],
                                        float(1 / (2 * PI)), 32.0, ALU.mult, ALU.add)
                xoff = wpool.tile([128, BL], DT, tag="sc_xoff")
                nc.vector.tensor_scalar(xoff[:], fd_ps[2 + half][:], float(64 * PI),
                                        None, ALU.add)
                y1 = wpool.tile([128, BL], DT, tag="sc_y1")
                nc.vector.scalar_tensor_tensor(y1[:], ki[:], float(-2 * PI), xoff[:],
                                               ALU.mult, ALU.add)
                w = wpool.tile([128, BL], DT, tag="sc_w")
                nc.vector.tensor_scalar(w[:], y1[:], PI, float(-2 * PI),
                                        ALU.is_gt, ALU.mult)
                y2 = wpool.tile([128, BL], DT, tag="sc_y2")
                nc.vector.tensor_tensor(y2[:], y1[:], w[:], ALU.add)
                y = wpool.tile([128, BL], DT, tag="sc_y")
                nc.vector.tensor_scalar(y[:], y2[:], -PI, PI, ALU.max, ALU.min)
                s = wpool.tile([128, BL], DT, tag=f"sinT{half}")
                nc.scalar.activation(s[:], y[:], ACTF.Sin)
                # cos(y) = sin(pi/2 - |y|),  argument stays in [-pi/2, pi/2]
                ay = wpool.tile([128, BL], DT, tag="sc_ay")
                nc.scalar.activation(ay[:], y[:], ACTF.Abs)
                c = wpool.tile([128, BL], DT, tag=f"cosT{half}")
                nc.scalar.activation(c[:], ay[:], ACTF.Sin, bias=bias_hpi[:],
                                     scale=-1.0)
                sinT.append(s)
                cosT.append(c)

            # ---- U = F @ cos, V = F @ sin ; dtT = sin*U - cos*V ----------
            dtT = []
            for m in range(2):
                psU = ppool.tile([128, BL], DT, tag="ps")
                psV = ppool.tile([128, BL], DT, tag="ps")
                for k in range(2):
                    lhs = stFTp[:, k * A + m * 128:k * A + (m + 1) * 128]
                    nc.tensor.matmul(psU[:], lhs, cosT[k][:], start=(k == 0), stop=(k == 1))
                    uv_last = nc.tensor.matmul(psV[:], lhs, sinT[k][:],
                                               start=(k == 0), stop=(k == 1))
                t1 = wpool.tile([128, BL], DT, tag="dt_t1")
                nc.vector.tensor_tensor(t1[:], sinT[m][:], psU[:], ALU.mult)
                t2 = wpool.tile([128, BL], DT, tag="dt_t2")
                nc.vector.tensor_tensor(t2[:], cosT[m][:], psV[:], ALU.mult)
                t = wpool.tile([128, BL], DT, tag=f"dtT{m}")
                nc.vector.tensor_tensor(t[:], t1[:], t2[:], ALU.subtract)
                dtT.append(t)

            # ---- new_state stacked matmul: everything that doesn't need
            # action accumulates into PSUM while the basis runs -------------
            ns_ps = ppool_ns.tile([BL, S], DT, tag="ns_ps")
            ns_first = nc.tensor.matmul(ns_ps[:], ones_row[:], st3[:],
                                        start=True, stop=False)
            tile.add_dep_helper(ns_first.ins, uv_last.ins, sync=False,
                                reason="keep PE free for U/V before the wns stack")
            for k in range(4):
                nc.tensor.matmul(ns_ps[:], prevT(k), wns(k), start=False, stop=False)
            for k in range(2):
                nc.tensor.matmul(ns_ps[:], inpp[:, k * BL:(k + 1) * BL], wns(6 + k),
                                 start=False, stop=False)
            for k in range(2):
                nc.tensor.matmul(ns_ps[:], dtT[k][:], wns(4 + k),
                                 start=False, stop=False)

            # ---- piecewise-linear basis (the fat part) -------------------
            # anc = W63p*r+ - sum_i wp_i*min(c_i,r+)
            #     + W63m*r- - sum_i wm_i*min(c'_i,r-)
            actp = wpool.tile([128, 2 * BL], DT, tag="actp")
            actT = []
            for half in range(2):
                tbr = []
                for (r, wb, cc, w63) in (
                    (r_p[half], wpb[half], cK[half], W63p[half]),
                    (r_m[half], wmb[half], cKm[half], W63m[half]),
                ):
                    r_bc = r[:].rearrange("p (b u) -> p b u", u=1).broadcast_to([128, BL, I])
                    c_bc = cc[:].rearrange("p (u i) -> p u i", u=1).broadcast_to([128, BL, I])
                    w_bc = wb[:].rearrange("p (u i) -> p u i", u=1).broadcast_to([128, BL, I])
                    M = fatpool.tile([128, BL * I], BF, tag="M")
                    M3 = M[:].rearrange("p (b i) -> p b i", i=I)
                    nc.vector.tensor_tensor(M3, r_bc, c_bc, ALU.min)
                    Y = fatpool.tile([128, BL * I], BF, tag="Y")
                    Y3 = Y[:].rearrange("p (b i) -> p b i", i=I)
                    nc.vector.tensor_tensor(Y3, M3, w_bc, ALU.mult)
                    F1 = fatpool.tile([128, BL * 32], BF, tag="F1")
                    F13 = F1[:].rearrange("p (b i) -> p b i", i=32)
                    nc.vector.tensor_tensor(F13, Y3[:, :, 0:32], Y3[:, :, 32:64], ALU.add)
                    F2 = fatpool.tile([128, BL * 16], BF, tag="F2")
                    F23 = F2[:].rearrange("p (b i) -> p b i", i=16)
                    nc.vector.tensor_tensor(F23, F13[:, :, 0:16], F13[:, :, 16:32], ALU.add)
                    red = wpool.tile([128, BL], DT, tag="red")
                    nc.vector.tensor_reduce(red[:], F23, AX.X, ALU.add)
                    t = wpool.tile([128, BL], DT, tag="tbr")
                    nc.vector.scalar_tensor_tensor(t[:], r[:], w63[:], red[:],
                                                   ALU.mult, ALU.subtract)
                    tbr.append(t)
                anc = wpool.tile([128, BL], DT, tag="anc")
                nc.vector.tensor_tensor(anc[:], tbr[0][:], tbr[1][:], ALU.add)
                at = actp[:, half * BL:(half + 1) * BL]
                nc.vector.tensor_scalar(at, anc[:], nma_col[:], ma_col,
                                        ALU.max, ALU.min)
                actT.append(at)
            nc.sync.dma_start(out=d_actp[:], in_=actp[:])

            # action rows of the stacked matmul (wns blocks 8,9 = st4)
            for k in range(2):
                nc.tensor.matmul(ns_ps[:], actT[k], wns(8 + k),
                                 start=False, stop=(k == 1))

            ns_nat = wpool.tile([BL, S], DT, tag="ns_nat")
            nc.scalar.activation(ns_nat[:], ns_ps[:], ACTF.Copy)
            nc.sync.dma_start(out=d_ns[:], in_=ns_nat[:])

            # ---- transpose new_state back to [s, b] for the out matmuls --
            nsT, ns2T = [], []
            for k in range(4):
                ps = ppool.tile([128, BL], DT, tag="ps")
                nc.tensor.transpose(ps[:], ns_nat[:, k * 128:(k + 1) * 128], ident[:])
                t = wpool.tile([128, BL], DT, tag=f"nsT{k}")
                nc.scalar.activation(t[:], ps[:], ACTF.Copy)
                nsT.append(t)
                t2 = wpool.tile([128, BL], DT, tag=f"ns2T{k}")
                nc.vector.tensor_tensor(t2[:], t[:], t[:], ALU.mult)
                ns2T.append(t2)

            # ---- frequency = ns @ select_w ; loss0 = ns^2 @ saw ----------
            fq_ps = ppool.tile([BL, A], DT, tag="ps")
            for k in range(4):
                nc.tensor.matmul(fq_ps[:], nsT[k][:], selc(k)[:, 0:A],
                                 start=(k == 0), stop=(k == 3))
            fq = wpool.tile([BL, A], DT, tag="fq")
            nc.scalar.activation(fq[:], fq_ps[:], ACTF.Copy)
            nc.scalar.dma_start(out=d_freq[:], in_=fq[:])

            ls_ps = ppool.tile([BL, 1], DT, tag="ps")
            for k in range(4):
                nc.tensor.matmul(ls_ps[:], ns2T[k][:], sawma[:, k:k + 1],
                                 start=(k == 0), stop=(k == 3))
            ls = wpool.tile([BL, 1], DT, tag="ls")
            nc.scalar.activation(ls[:], ls_ps[:], ACTF.Copy)
            nc.scalar.dma_start(out=d_loss[:], in_=ls[:])

    nc.compile()
    return nc


_NC_CACHE = None


def _get_nc():
    global _NC_CACHE
    if _NC_CACHE is None:
        _NC_CACHE = build_nc()
    return _NC_CACHE


def _pack_rows(a, width):
    """[R, C] with R = 128*n  ->  [128, n*C] panel (blocks along free dim)."""
    r, c = a.shape
    n = r // 128
    return np.ascontiguousarray(
        a.reshape(n, 128, c).transpose(1, 0, 2).reshape(128, n * c))


def prepare_in_maps(inputs):
    x = {k: np.asarray(v) for k, v in inputs.items()}
    f32 = lambda a: np.ascontiguousarray(a, dtype=np.float32)

    sel_cat = np.concatenate([x["select_w"], x["select_delta"]], axis=1)
    wns = np.concatenate([x["state_transfer1"], x["state_transfer2"],
                          x["state_transfer3_Pm"], x["state_transfer4"]], axis=0)
    sawma = np.zeros((128, 5), np.float32)
    sawma[:, 0:4] = x["select_add_w"].reshape(4, 128).T
    sawma[:, 4] = np.float32(x["max_action"])

    shared = {
        "bp4": f32(np.concatenate([x["w_plus_temp0"].T, x["b_plus_temp0"].T,
                                   x["w_minus_temp0"].T, x["b_minus_temp0"].T],
                                  axis=1)),
        "rec2": f32(np.concatenate([x["w_recover"], x["b_recover"]], axis=1)),
        "selcatp": f32(_pack_rows(sel_cat, 2 * A)),
        "sawma": sawma,
        "stFTp": f32(_pack_rows(x["state_transferF"].T, A)),
        "st3": f32(x["state_transfer3"][None, :]),
        "wnsa": f32(_pack_rows(wns[:640], S)),
        "wnsb": f32(_pack_rows(wns[640:], S)),
    }
    in_maps = []
    for k in range(NCORES):
        sl = slice(k * BL, (k + 1) * BL)
        m = dict(shared)
        m["prevp"] = f32(_pack_rows(x["prev_output"][sl].T, BL))
        m["inpp"] = f32(_pack_rows(x["inputs"][sl].T, BL))
        in_maps.append(m)
    return in_maps


def postprocess(res):
    loss0 = np.concatenate([res[k]["loss_out"] for k in range(NCORES)], axis=0)
    frequency = np.concatenate([res[k]["freq_out"] for k in range(NCORES)], axis=0)
    action = np.concatenate(
        [np.concatenate([res[k]["actp_out"][:, 0:BL],
                         res[k]["actp_out"][:, BL:2 * BL]], axis=0).T
         for k in range(NCORES)], axis=0)
    new_state = np.concatenate([res[k]["ns_out"] for k in range(NCORES)], axis=0)
    return (loss0, frequency, action, new_state)


def kernel(**inputs):
    in_maps = prepare_in_maps(inputs)
    res = run_bass_kernel_spmd(_get_nc(), in_maps, list(range(NCORES))).results
    return postprocess(res)


# revision 13
# speedup vs baseline: 1.1301x; 1.0033x over previous
"""Trainium2 Bass kernel for nn_CustomRNNCell (Kuramoto-style RNN cell).

Strategy: pure data parallelism over the batch dim (B=512 -> 64 rows/core,
8 cores), parameters replicated.  All activations live on-chip in a
"transposed" layout [feature, batch] so every weight matrix is consumed by
the PE untransposed; the host does the input transposes / output
un-transposes / tensor packing (pure data movement).

Key algebraic restructurings (validated against the reference):
  * delta_term = sin(d)*(F @ cos(d)) - cos(d)*(F @ sin(d))   (angle-difference
    expansion; kills the [B,A,A] sin grid)
  * piecewise-linear basis: with c = cumsum(b_t0^2) (knots, increasing),
      sum_i w_i*relu(f - c_i) = W63*relu(f) - sum_i w_i*min(c_i, relu(f))
    (the sum-w*c constants cancel between the two relu branches), computed
    with broadcast access patterns on the DVE; the fat min/mult/fold passes
    only ever see values <= c_max ~ 0.6 so they run in bf16, while the
    dominant W63*relu(f) term stays fp32.
  * clip(x,-m,m) = min(max(x,-m),m) as one tensor_scalar op.
  * new_state via one PSUM-accumulated matmul over the stacked
    [1; prev; inputs; delta_term; action] x [st3; st1; st3_Pm; st2; st4];
    everything not needing `action` accumulates while the basis runs.
  * params are host-packed into a handful of [128, N] panels -> one DMA each
    (the HWDGE ring serializes DMAs at ~0.6us apiece).
"""

import sys

for _p in ("/opt/trn_rl_repo",):
    if _p not in sys.path:
        sys.path.insert(0, _p)

import numpy as np

import concourse.bacc as bacc
import concourse.mybir as mybir
import concourse.tile as tile
from concourse.bass_utils import run_bass_kernel_spmd
from concourse.masks import make_identity

B, A, I = 512, 256, 64
S, P = 512, 256
NCORES = 8
BL = B // NCORES  # 64 batch rows per core

DT = mybir.dt.float32
BF = mybir.dt.bfloat16
AX = mybir.AxisListType
ALU = mybir.AluOpType
ACTF = mybir.ActivationFunctionType

PI = float(np.pi)


def build_nc():
    nc = bacc.Bacc()

    # ---- DRAM I/O (host-packed panels) -------------------------------
    d_bp4 = nc.dram_tensor("bp4", [I, 4 * A], DT, kind="ExternalInput")
    d_rec2 = nc.dram_tensor("rec2", [I, 2 * I], DT, kind="ExternalInput")
    d_prevp = nc.dram_tensor("prevp", [128, 4 * BL], DT, kind="ExternalInput")
    d_selcat = nc.dram_tensor("selcatp", [128, 4 * 2 * A], DT, kind="ExternalInput")
    d_sawma = nc.dram_tensor("sawma", [128, 5], DT, kind="ExternalInput")
    d_stFTp = nc.dram_tensor("stFTp", [128, 2 * A], DT, kind="ExternalInput")
    d_inpp = nc.dram_tensor("inpp", [128, 2 * BL], DT, kind="ExternalInput")
    d_st3 = nc.dram_tensor("st3", [1, S], DT, kind="ExternalInput")
    d_wnsa = nc.dram_tensor("wnsa", [128, 5 * S], DT, kind="ExternalInput")
    d_wnsb = nc.dram_tensor("wnsb", [128, 5 * S], DT, kind="ExternalInput")

    d_ns = nc.dram_tensor("ns_out", [BL, S], DT, kind="ExternalOutput")
    d_freq = nc.dram_tensor("freq_out", [BL, A], DT, kind="ExternalOutput")
    d_loss = nc.dram_tensor("loss_out", [BL, 1], DT, kind="ExternalOutput")
    d_actp = nc.dram_tensor("actp_out", [128, 2 * BL], DT, kind="ExternalOutput")

    with tile.TileContext(nc) as tc:
        with (
            tc.tile_pool(name="const", bufs=1) as cpool,
            tc.tile_pool(name="work", bufs=2) as wpool,
            tc.tile_pool(name="fat", bufs=2) as fatpool,
            tc.tile_pool(name="psum", bufs=6, space="PSUM") as ppool,
            tc.tile_pool(name="psum_ns", bufs=1, space="PSUM") as ppool_ns,
        ):
            # ---- input DMAs: one per panel, in order of need -----------
            bp4 = cpool.tile([I, 4 * A], DT, tag="bp4")
            nc.sync.dma_start(out=bp4[:], in_=d_bp4[:])
            rec2 = cpool.tile([I, 2 * I], DT, tag="rec2")
            nc.sync.dma_start(out=rec2[:], in_=d_rec2[:])
            prevp = cpool.tile([128, 4 * BL], DT, tag="prevp")
            nc.sync.dma_start(out=prevp[:], in_=d_prevp[:])
            selcat = cpool.tile([128, 4 * 2 * A], DT, tag="selcat")
            nc.sync.dma_start(out=selcat[:], in_=d_selcat[:])
            wnsa = cpool.tile([128, 5 * S], DT, tag="wnsa")
            nc.sync.dma_start(out=wnsa[:], in_=d_wnsa[:])
            # the rest goes on the ACT HWDGE ring, in parallel
            sawma = cpool.tile([128, 5], DT, tag="sawma")
            nc.scalar.dma_start(out=sawma[:], in_=d_sawma[:])
            stFTp = cpool.tile([128, 2 * A], DT, tag="stFTp")
            nc.scalar.dma_start(out=stFTp[:], in_=d_stFTp[:])
            inpp = cpool.tile([128, 2 * BL], DT, tag="inpp")
            nc.scalar.dma_start(out=inpp[:], in_=d_inpp[:])
            st3 = cpool.tile([1, S], DT, tag="st3")
            nc.scalar.dma_start(out=st3[:], in_=d_st3[:])
            wnsb = cpool.tile([128, 5 * S], DT, tag="wnsb")
            nc.scalar.dma_start(out=wnsb[:], in_=d_wnsb[:])

            def prevT(k):
                return prevp[:, k * BL:(k + 1) * BL]

            def selc(k):  # [128, 512] K-tile of [select_w | select_delta]
                return selcat[:, k * 2 * A:(k + 1) * 2 * A]

            def wns(k):
                if k < 5:
                    return wnsa[:, k * S:(k + 1) * S]
                return wnsb[:, (k - 5) * S:(k - 5 + 1) * S]

            ident = cpool.tile([BL, BL], DT, tag="ident")
            make_identity(nc, ident[:])
            ones_row = cpool.tile([1, BL], DT, tag="ones_row")
            nc.vector.memset(ones_row[:], 1.0)
            ma_col = sawma[:, 4:5]
            nma_col = cpool.tile([128, 1], DT, tag="nma")
            nc.vector.tensor_scalar(nma_col[:], ma_col, -1.0, None, ALU.mult)
            bias_hpi = cpool.tile([128, 1], DT, tag="bias_hpi")
            nc.vector.memset(bias_hpi[:], PI / 2)

            # ---- param prep: squares (DVE; avoids the ACT Square-table
            # load) + small matmuls ---------------------------------------
            sq = {}
            for j, nm in enumerate(("wp", "bp", "wm", "bm")):
                t = wpool.tile([I, A], DT, tag=f"sq_{nm}")
                src = bp4[:, j * A:(j + 1) * A]
                nc.vector.tensor_tensor(t[:], src, src, ALU.mult)
                sq[nm] = t
            wrec = rec2[:, 0:I]
            brec = rec2[:, I:2 * I]

            # w_plus = wp2 @ wrec ; w_minus = -(wm2 @ wrec)
            # c = bp2 @ brec ; c' = bm2 @ brec   (knots; = -b_plus / -b_minus)
            # w in bf16 for the fat passes; W63 = sum_i w_i in fp32 from PSUM.
            wpb, wmb, cK, cKm, W63p, W63m = [], [], [], [], [], []
            for half in range(2):
                ms = slice(half * 128, (half + 1) * 128)
                for nm, rhs, neg in (("wp", wrec, False), ("wm", wrec, True),
                                     ("bp", brec, False), ("bm", brec, False)):
                    ps = ppool.tile([128, I], DT, tag="ps")
                    nc.tensor.matmul(ps[:], sq[nm][:, ms], rhs, start=True, stop=True)
                    if nm in ("wp", "wm"):
                        t = wpool.tile([128, I], BF, tag=f"{nm}_{half}")
                        nc.scalar.activation(t[:], ps[:], ACTF.Copy,
                                             scale=-1.0 if neg else 1.0)
                        (wpb if nm == "wp" else wmb).append(t)
                        w63 = wpool.tile([128, 1], DT, tag=f"w63_{nm}{half}")
                        nc.vector.tensor_reduce(w63[:], ps[:], AX.X, ALU.add,
                                                negate=neg)
                        (W63p if nm == "wp" else W63m).append(w63)
                    else:
                        t = wpool.tile([128, I], DT, tag=f"{nm}_{half}")
                        nc.scalar.activation(t[:], ps[:], ACTF.Copy)
                        (cK if nm == "bp" else cKm).append(t)

            # ---- freq / delta:  fdT = sel_cat^T-as-lhsT @ prevT ----------
            fd_ps = []
            for m in range(4):
                ps = ppool.tile([128, BL], DT, tag="ps")
                for k in range(4):
                    nc.tensor.matmul(ps[:], selc(k)[:, m * 128:(m + 1) * 128],
                                     prevT(k), start=(k == 0), stop=(k == 3))
                fd_ps.append(ps)

            # r+ = relu(f), r- = relu(-f)  (fp32, straight from PSUM)
            r_p, r_m = [], []
            for half in range(2):
                rp = wpool.tile([128, BL], DT, tag=f"r_p{half}")
                nc.vector.tensor_scalar(rp[:], fd_ps[half][:], 0.0, None, ALU.max)
                rm = wpool.tile([128, BL], DT, tag=f"r_m{half}")
                nc.vector.tensor_scalar(rm[:], fd_ps[half][:], -1.0, 0.0,
                                        ALU.mult, ALU.max)
                r_p.append(rp)
                r_m.append(rm)

            # ---- basis fat-pass helper ----------------------------------
            def fat_unit(r, wb, cc, w63):
                r_bc = r[:].rearrange("p (b u) -> p b u", u=1).broadcast_to([128, BL, I])
                c_bc = cc[:].rearrange("p (u i) -> p u i", u=1).broadcast_to([128, BL, I])
                w_bc = wb[:].rearrange("p (u i) -> p u i", u=1).broadcast_to([128, BL, I])
                M = fatpool.tile([128, BL * I], BF, tag="M")
                M3 = M[:].rearrange("p (b i) -> p b i", i=I)
                nc.vector.tensor_tensor(M3, r_bc, c_bc, ALU.min)
                Y = fatpool.tile([128, BL * I], BF, tag="Y")
                Y3 = Y[:].rearrange("p (b i) -> p b i", i=I)
                nc.vector.tensor_tensor(Y3, M3, w_bc, ALU.mult)
                F1 = fatpool.tile([128, BL * 32], BF, tag="F1")
                F13 = F1[:].rearrange("p (b i) -> p b i", i=32)
                nc.vector.tensor_tensor(F13, Y3[:, :, 0:32], Y3[:, :, 32:64], ALU.add)
                F2 = fatpool.tile([128, BL * 16], BF, tag="F2")
                F23 = F2[:].rearrange("p (b i) -> p b i", i=16)
                nc.vector.tensor_tensor(F23, F13[:, :, 0:16], F13[:, :, 16:32], ALU.add)
                red = wpool.tile([128, BL], DT, tag="red")
                nc.vector.tensor_reduce(red[:], F23, AX.X, ALU.add)
                t = wpool.tile([128, BL], DT, tag="tbr")
                nc.vector.scalar_tensor_tensor(t[:], r[:], w63[:], red[:],
                                               ALU.mult, ALU.subtract)
                return t

            actp = wpool.tile([128, 2 * BL], DT, tag="actp")
            actT = []

            def finish_half(half, tp, tm):
                anc = wpool.tile([128, BL], DT, tag="anc")
                nc.vector.tensor_tensor(anc[:], tp[:], tm[:], ALU.add)
                at = actp[:, half * BL:(half + 1) * BL]
                nc.vector.tensor_scalar(at, anc[:], nma_col[:], ma_col,
                                        ALU.max, ALU.min)
                actT.append(at)

            # FAT unit (+,0) goes first so the DVE gets busy asap
            t_p0 = fat_unit(r_p[0], wpb[0], cK[0], W63p[0])

            # ---- sin / cos of delta (range-reduced; fills DVE/ACT gaps) --
            # y = x - 2pi*k via an int32 cast (round-to-nearest on HW,
            # trunc in CoreSim); a branch-free +-2pi correction makes the
            # result [-pi, pi] under either conversion mode.
            sinT, cosT = [], []
            for half in range(2):
                ki = wpool.tile([128, BL], mybir.dt.int32, tag="sc_ki")
                nc.vector.tensor_scalar(ki[:], fd_ps[2 + half][:],
                                        float(1 / (2 * PI)), 32.0, ALU.mult, ALU.add)
                xoff = wpool.tile([128, BL], DT, tag="sc_xoff")
                nc.vector.tensor_scalar(xoff[:], fd_ps[2 + half][:], float(64 * PI),
                                        None, ALU.add)
                y1 = wpool.tile([128, BL], DT, tag="sc_y1")
                nc.vector.scalar_tensor_tensor(y1[:], ki[:], float(-2 * PI), xoff[:],
                                               ALU.mult, ALU.add)
                w = wpool.tile([128, BL], DT, tag="sc_w")
                nc.vector.tensor_scalar(w[:], y1[:], PI, float(-2 * PI),
                                        ALU.is_gt, ALU.mult)
                y2 = wpool.tile([128, BL], DT, tag="sc_y2")
                nc.vector.tensor_tensor(y2[:], y1[:], w[:], ALU.add)
                y = wpool.tile([128, BL], DT, tag="sc_y")
                nc.vector.tensor_scalar(y[:], y2[:], -PI, PI, ALU.max, ALU.min)
                s = wpool.tile([128, BL], DT, tag=f"sinT{half}")
                nc.scalar.activation(s[:], y[:], ACTF.Sin)
                # cos(y) = sin(pi/2 - |y|),  argument stays in [-pi/2, pi/2]
                ay = wpool.tile([128, BL], DT, tag="sc_ay")
                nc.scalar.activation(ay[:], y[:], ACTF.Abs)
                c = wpool.tile([128, BL], DT, tag=f"cosT{half}")
                nc.scalar.activation(c[:], ay[:], ACTF.Sin, bias=bias_hpi[:],
                                     scale=-1.0)
                sinT.append(s)
                cosT.append(c)

            t_m0 = fat_unit(r_m[0], wmb[0], cKm[0], W63m[0])
            finish_half(0, t_p0, t_m0)

            # ---- U = F @ cos, V = F @ sin ; dtT = sin*U - cos*V ----------
            dtT = []
            for m in range(2):
                psU = ppool.tile([128, BL], DT, tag="ps")
                psV = ppool.tile([128, BL], DT, tag="ps")
                for k in range(2):
                    lhs = stFTp[:, k * A + m * 128:k * A + (m + 1) * 128]
                    nc.tensor.matmul(psU[:], lhs, cosT[k][:], start=(k == 0), stop=(k == 1))
                    uv_last = nc.tensor.matmul(psV[:], lhs, sinT[k][:],
                                               start=(k == 0), stop=(k == 1))
                t1 = wpool.tile([128, BL], DT, tag="dt_t1")
                nc.vector.tensor_tensor(t1[:], sinT[m][:], psU[:], ALU.mult)
                t2 = wpool.tile([128, BL], DT, tag="dt_t2")
                nc.vector.tensor_tensor(t2[:], cosT[m][:], psV[:], ALU.mult)
                t = wpool.tile([128, BL], DT, tag=f"dtT{m}")
                nc.vector.tensor_tensor(t[:], t1[:], t2[:], ALU.subtract)
                dtT.append(t)

            # ---- new_state stacked matmul: everything that doesn't need
            # action accumulates into PSUM while the basis runs -------------
            ns_ps = ppool_ns.tile([BL, S], DT, tag="ns_ps")
            ns_first = nc.tensor.matmul(ns_ps[:], ones_row[:], st3[:],
                                        start=True, stop=False)
            tile.add_dep_helper(ns_first.ins, uv_last.ins, sync=False,
                                reason="keep PE free for U/V before the wns stack")
            for k in range(4):
                nc.tensor.matmul(ns_ps[:], prevT(k), wns(k), start=False, stop=False)
            for k in range(2):
                nc.tensor.matmul(ns_ps[:], inpp[:, k * BL:(k + 1) * BL], wns(6 + k),
                                 start=False, stop=False)
            for k in range(2):
                nc.tensor.matmul(ns_ps[:], dtT[k][:], wns(4 + k),
                                 start=False, stop=False)
            # action half 0 as soon as it exists (wns block 8 = st4 rows 0:128)
            nc.tensor.matmul(ns_ps[:], actT[0], wns(8), start=False, stop=False)

            # remaining fat units
            t_p1 = fat_unit(r_p[1], wpb[1], cK[1], W63p[1])
            t_m1 = fat_unit(r_m[1], wmb[1], cKm[1], W63m[1])
            finish_half(1, t_p1, t_m1)
            nc.sync.dma_start(out=d_actp[:], in_=actp[:])
            nc.tensor.matmul(ns_ps[:], actT[1], wns(9), start=False, stop=True)

            ns_nat = wpool.tile([BL, S], DT, tag="ns_nat")
            nc.scalar.activation(ns_nat[:], ns_ps[:], ACTF.Copy)
            nc.sync.dma_start(out=d_ns[:], in_=ns_nat[:])

            # ---- transpose new_state back to [s, b] for the out matmuls --
            nsT, ns2T = [], []
            for k in range(4):
                ps = ppool.tile([128, BL], DT, tag="ps")
                nc.tensor.transpose(ps[:], ns_nat[:, k * 128:(k + 1) * 128], ident[:])
                t = wpool.tile([128, BL], DT, tag=f"nsT{k}")
                nc.scalar.activation(t[:], ps[:], ACTF.Copy)
                nsT.append(t)
                t2 = wpool.tile([128, BL], DT, tag=f"ns2T{k}")
                nc.vector.tensor_tensor(t2[:], t[:], t[:], ALU.mult)
                ns2T.append(t2)

            # ---- frequency = ns @ select_w ; loss0 = ns^2 @ saw ----------
            fq_ps = ppool.tile([BL, A], DT, tag="ps")
            for k in range(4):
                nc.tensor.matmul(fq_ps[:], nsT[k][:], selc(k)[:, 0:A],
                                 start=(k == 0), stop=(k == 3))
            fq = wpool.tile([BL, A], DT, tag="fq")
            nc.scalar.activation(fq[:], fq_ps[:], ACTF.Copy)
            nc.sync.dma_start(out=d_freq[:], in_=fq[:])

            ls_ps = ppool.tile([BL, 1], DT, tag="ps")
            for k in range(4):
                nc.tensor.matmul(ls_ps[:], ns2T[k][:], sawma[:, k:k + 1],
                                 start=(k == 0), stop=(k == 3))
            ls = wpool.tile([BL, 1], DT, tag="ls")
            nc.scalar.activation(ls[:], ls_ps[:], ACTF.Copy)
            nc.sync.dma_start(out=d_loss[:], in_=ls[:])

    nc.compile()
    return nc


_NC_CACHE = None


def _get_nc():
    global _NC_CACHE
    if _NC_CACHE is None:
        _NC_CACHE = build_nc()
    return _NC_CACHE


def _pack_rows(a, width):
    """[R, C] with R = 128*n  ->  [128, n*C] panel (blocks along free dim)."""
    r, c = a.shape
    n = r // 128
    return np.ascontiguousarray(
        a.reshape(n, 128, c).transpose(1, 0, 2).reshape(128, n * c))


def prepare_in_maps(inputs):
    x = {k: np.asarray(v) for k, v in inputs.items()}
    f32 = lambda a: np.ascontiguousarray(a, dtype=np.float32)

    sel_cat = np.concatenate([x["select_w"], x["select_delta"]], axis=1)
    wns = np.concatenate([x["state_transfer1"], x["state_transfer2"],
                          x["state_transfer3_Pm"], x["state_transfer4"]], axis=0)
    sawma = np.zeros((128, 5), np.float32)
    sawma[:, 0:4] = x["select_add_w"].reshape(4, 128).T
    sawma[:, 4] = np.float32(x["max_action"])

    shared = {
        "bp4": f32(np.concatenate([x["w_plus_temp0"].T, x["b_plus_temp0"].T,
                                   x["w_minus_temp0"].T, x["b_minus_temp0"].T],
                                  axis=1)),
        "rec2": f32(np.concatenate([x["w_recover"], x["b_recover"]], axis=1)),
        "selcatp": f32(_pack_rows(sel_cat, 2 * A)),
        "sawma": sawma,
        "stFTp": f32(_pack_rows(x["state_transferF"].T, A)),
        "st3": f32(x["state_transfer3"][None, :]),
        "wnsa": f32(_pack_rows(wns[:640], S)),
        "wnsb": f32(_pack_rows(wns[640:], S)),
    }
    in_maps = []
    for k in range(NCORES):
        sl = slice(k * BL, (k + 1) * BL)
        m = dict(shared)
        m["prevp"] = f32(_pack_rows(x["prev_output"][sl].T, BL))
        m["inpp"] = f32(_pack_rows(x["inputs"][sl].T, BL))
        in_maps.append(m)
    return in_maps


def postprocess(res):
    loss0 = np.concatenate([res[k]["loss_out"] for k in range(NCORES)], axis=0)
    frequency = np.concatenate([res[k]["freq_out"] for k in range(NCORES)], axis=0)
    action = np.concatenate(
        [np.concatenate([res[k]["actp_out"][:, 0:BL],
                         res[k]["actp_out"][:, BL:2 * BL]], axis=0).T
         for k in range(NCORES)], axis=0)
    new_state = np.concatenate([res[k]["ns_out"] for k in range(NCORES)], axis=0)
    return (loss0, frequency, action, new_state)


def kernel(**inputs):
    in_maps = prepare_in_maps(inputs)
    res = run_bass_kernel_spmd(_get_nc(), in_maps, list(range(NCORES))).results
    return postprocess(res)


# revision 14
# speedup vs baseline: 1.1862x; 1.0497x over previous
"""Trainium2 Bass kernel for nn_CustomRNNCell (Kuramoto-style RNN cell).

Strategy: pure data parallelism over the batch dim (B=512 -> 64 rows/core,
8 cores), parameters replicated.  All activations live on-chip in a
"transposed" layout [feature, batch] so every weight matrix is consumed by
the PE untransposed; the host does the input transposes / output
un-transposes / tensor packing (pure data movement).

Key algebraic restructurings (validated against the reference):
  * w_recover / b_recover are difference / strict-cumsum operators, so the
    basis-parameter prep is a square + shifted subtract + prefix scan on the
    DVE (no matmuls), and W63 = sum_i w_i is just the last column of wp^2.
  * delta_term = sin(d)*(F @ cos(d)) - cos(d)*(F @ sin(d))   (angle-difference
    expansion; kills the [B,A,A] sin grid)
  * piecewise-linear basis: with c = cumsum(b_t0^2) (knots, increasing),
      sum_i w_i*relu(f - c_i) = W63*relu(f) - sum_i w_i*min(c_i, relu(f))
    (the sum-w*c constants cancel between the two relu branches), computed
    with broadcast access patterns on the DVE; the fat min/mult/fold passes
    only ever see values <= c_max ~ 0.6 so they run in bf16, while the
    dominant W63*relu(f) term stays fp32.
  * clip(x,-m,m) = min(max(x,-m),m) as one tensor_scalar op.
  * new_state via one PSUM-accumulated matmul over the stacked
    [1; prev; inputs; delta_term; action] x [st3; st1; st3_Pm; st2; st4];
    everything not needing `action` accumulates while the basis runs.
  * params are host-packed into a handful of [128, N] panels -> one DMA each
    (the HWDGE ring serializes DMAs at ~0.6us apiece), split across the two
    HWDGE rings (sync + scalar).
"""

import sys

for _p in ("/opt/trn_rl_repo",):
    if _p not in sys.path:
        sys.path.insert(0, _p)

import numpy as np

import concourse.bacc as bacc
import concourse.mybir as mybir
import concourse.tile as tile
from concourse.bass_utils import run_bass_kernel_spmd
from concourse.masks import make_identity

B, A, I = 512, 256, 64
S, P = 512, 256
NCORES = 8
BL = B // NCORES  # 64 batch rows per core

DT = mybir.dt.float32
BF = mybir.dt.bfloat16
AX = mybir.AxisListType
ALU = mybir.AluOpType
ACTF = mybir.ActivationFunctionType

PI = float(np.pi)


def build_nc():
    nc = bacc.Bacc()

    # ---- DRAM I/O (host-packed panels) -------------------------------
    # pk: [128, (half, param, i)] = natural-layout wp_t0|bp_t0|wm_t0|bm_t0
    d_pk = nc.dram_tensor("pk", [128, 2 * 4 * I], DT, kind="ExternalInput")
    d_prevp = nc.dram_tensor("prevp", [128, 4 * BL], DT, kind="ExternalInput")
    d_selw = nc.dram_tensor("selwp", [128, 4 * A], DT, kind="ExternalInput")
    d_seld = nc.dram_tensor("seldp", [128, 4 * A], DT, kind="ExternalInput")
    d_sawma = nc.dram_tensor("sawma", [128, 5], DT, kind="ExternalInput")
    d_stFTp = nc.dram_tensor("stFTp", [128, 2 * A], DT, kind="ExternalInput")
    d_inpp = nc.dram_tensor("inpp", [128, 2 * BL], DT, kind="ExternalInput")
    d_st3 = nc.dram_tensor("st3", [1, S], DT, kind="ExternalInput")
    d_wnsa = nc.dram_tensor("wnsa", [128, 5 * S], DT, kind="ExternalInput")
    d_wnsb = nc.dram_tensor("wnsb", [128, 5 * S], DT, kind="ExternalInput")

    d_ns = nc.dram_tensor("ns_out", [BL, S], DT, kind="ExternalOutput")
    d_freq = nc.dram_tensor("freq_out", [BL, A], DT, kind="ExternalOutput")
    d_loss = nc.dram_tensor("loss_out", [BL, 1], DT, kind="ExternalOutput")
    d_actp = nc.dram_tensor("actp_out", [128, 2 * BL], DT, kind="ExternalOutput")

    with tile.TileContext(nc) as tc:
        with (
            tc.tile_pool(name="const", bufs=1) as cpool,
            tc.tile_pool(name="work", bufs=2) as wpool,
            tc.tile_pool(name="fat", bufs=2) as fatpool,
            tc.tile_pool(name="psum", bufs=6, space="PSUM") as ppool,
            tc.tile_pool(name="psum_ns", bufs=1, space="PSUM") as ppool_ns,
        ):
            # ---- input DMAs: one per panel, in order of need -----------
            pk = cpool.tile([128, 2 * 4 * I], DT, tag="pk")
            nc.sync.dma_start(out=pk[:], in_=d_pk[:])
            prevp = cpool.tile([128, 4 * BL], DT, tag="prevp")
            nc.sync.dma_start(out=prevp[:], in_=d_prevp[:])
            selw = cpool.tile([128, 4 * A], DT, tag="selw")
            nc.sync.dma_start(out=selw[:], in_=d_selw[:])
            wnsa = cpool.tile([128, 5 * S], DT, tag="wnsa")
            nc.sync.dma_start(out=wnsa[:], in_=d_wnsa[:])
            # the rest goes on the ACT HWDGE ring, in parallel
            sawma = cpool.tile([128, 5], DT, tag="sawma")
            nc.scalar.dma_start(out=sawma[:], in_=d_sawma[:])
            seld = cpool.tile([128, 4 * A], DT, tag="seld")
            nc.scalar.dma_start(out=seld[:], in_=d_seld[:])
            stFTp = cpool.tile([128, 2 * A], DT, tag="stFTp")
            nc.scalar.dma_start(out=stFTp[:], in_=d_stFTp[:])
            inpp = cpool.tile([128, 2 * BL], DT, tag="inpp")
            nc.scalar.dma_start(out=inpp[:], in_=d_inpp[:])
            st3 = cpool.tile([1, S], DT, tag="st3")
            nc.scalar.dma_start(out=st3[:], in_=d_st3[:])
            wnsb = cpool.tile([128, 5 * S], DT, tag="wnsb")
            nc.scalar.dma_start(out=wnsb[:], in_=d_wnsb[:])

            def prevT(k):
                return prevp[:, k * BL:(k + 1) * BL]

            def selwk(k):  # [128, 256] K-tile of select_w
                return selw[:, k * A:(k + 1) * A]

            def seldk(k):
                return seld[:, k * A:(k + 1) * A]

            def wns(k):
                if k < 5:
                    return wnsa[:, k * S:(k + 1) * S]
                return wnsb[:, (k - 5) * S:(k - 5 + 1) * S]

            ident = cpool.tile([BL, BL], DT, tag="ident")
            make_identity(nc, ident[:])
            ones_row = cpool.tile([1, BL], DT, tag="ones_row")
            nc.vector.memset(ones_row[:], 1.0)
            ma_col = sawma[:, 4:5]
            nma_col = cpool.tile([128, 1], DT, tag="nma")
            nc.vector.tensor_scalar(nma_col[:], ma_col, -1.0, None, ALU.mult)
            bias_hpi = cpool.tile([128, 1], DT, tag="bias_hpi")
            nc.vector.memset(bias_hpi[:], PI / 2)

            # ---- param prep, all on the DVE ------------------------------
            # sq2 = pk^2 ; per (half, param) slices are [128, 64]
            sq2 = wpool.tile([128, 2 * 4 * I], DT, tag="sq2")
            nc.vector.tensor_tensor(sq2[:], pk[:], pk[:], ALU.mult)

            def sqs(half, j):
                o = half * 4 * I + j * I
                return sq2[:, o:o + I], sq2[:, o + I - 1:o + I]  # slice, last col

            # w_plus = diff(wp2) (bf16), "wmb" = +diff(wm2) = -w_minus (bf16),
            # c = exclusive-cumsum(bp2), c' = exclusive-cumsum(bm2) (fp32)
            wpb, wmb, cK, cKm, W63p, W63m = [], [], [], [], [], []
            for half in range(2):
                wp2, wp2last = sqs(half, 0)
                bp2, _ = sqs(half, 1)
                wm2, wm2last = sqs(half, 2)
                bm2, _ = sqs(half, 3)
                W63p.append(wp2last)
                W63m.append(wm2last)  # positive; sign handled in finish_half

                t = wpool.tile([128, I], BF, tag=f"wpb{half}")
                nc.vector.tensor_copy(t[:, 0:1], wp2[:, 0:1])
                nc.vector.tensor_tensor(t[:, 1:I], wp2[:, 1:I], wp2[:, 0:I - 1],
                                        ALU.subtract)
                wpb.append(t)
                t = wpool.tile([128, I], BF, tag=f"wmb{half}")
                nc.vector.tensor_copy(t[:, 0:1], wm2[:, 0:1])
                nc.vector.tensor_tensor(t[:, 1:I], wm2[:, 1:I], wm2[:, 0:I - 1],
                                        ALU.subtract)
                wmb.append(t)
                for src2, lst in ((bp2, cK), (bm2, cKm)):
                    s = wpool.tile([128, I], DT, tag="scan")
                    nc.vector.tensor_tensor_scan(s[:], src2, src2, 0.0,
                                                 ALU.add, ALU.bypass)
                    cc = wpool.tile([128, I], DT, tag=f"c{len(lst)}_{half}")
                    nc.vector.tensor_tensor(cc[:], s[:], src2, ALU.subtract)
                    lst.append(cc)

            # ---- freq / delta:  fdT = sel^T-as-lhsT @ prevT --------------
            fd_ps = []
            for m in range(4):
                ps = ppool.tile([128, BL], DT, tag="ps")
                for k in range(4):
                    lhs = (selwk(k) if m < 2 else seldk(k))
                    mm = m % 2
                    nc.tensor.matmul(ps[:], lhs[:, mm * 128:(mm + 1) * 128],
                                     prevT(k), start=(k == 0), stop=(k == 3))
                fd_ps.append(ps)

            # r+ = relu(f), r- = relu(-f)  (fp32, straight from PSUM)
            r_p, r_m = [], []
            for half in range(2):
                rp = wpool.tile([128, BL], DT, tag=f"r_p{half}")
                nc.vector.tensor_scalar(rp[:], fd_ps[half][:], 0.0, None, ALU.max)
                rm = wpool.tile([128, BL], DT, tag=f"r_m{half}")
                nc.vector.tensor_scalar(rm[:], fd_ps[half][:], -1.0, 0.0,
                                        ALU.mult, ALU.max)
                r_p.append(rp)
                r_m.append(rm)

            # ---- basis fat-pass helper ----------------------------------
            def fat_unit(r, wb, cc, w63):
                r_bc = r[:].rearrange("p (b u) -> p b u", u=1).broadcast_to([128, BL, I])
                c_bc = cc[:].rearrange("p (u i) -> p u i", u=1).broadcast_to([128, BL, I])
                w_bc = wb[:].rearrange("p (u i) -> p u i", u=1).broadcast_to([128, BL, I])
                M = fatpool.tile([128, BL * I], BF, tag="M")
                M3 = M[:].rearrange("p (b i) -> p b i", i=I)
                nc.vector.tensor_tensor(M3, r_bc, c_bc, ALU.min)
                Y = fatpool.tile([128, BL * I], BF, tag="Y")
                Y3 = Y[:].rearrange("p (b i) -> p b i", i=I)
                nc.vector.tensor_tensor(Y3, M3, w_bc, ALU.mult)
                F1 = fatpool.tile([128, BL * 32], BF, tag="F1")
                F13 = F1[:].rearrange("p (b i) -> p b i", i=32)
                nc.vector.tensor_tensor(F13, Y3[:, :, 0:32], Y3[:, :, 32:64], ALU.add)
                F2 = fatpool.tile([128, BL * 16], BF, tag="F2")
                F23 = F2[:].rearrange("p (b i) -> p b i", i=16)
                nc.vector.tensor_tensor(F23, F13[:, :, 0:16], F13[:, :, 16:32], ALU.add)
                red = wpool.tile([128, BL], DT, tag="red")
                nc.vector.tensor_reduce(red[:], F23, AX.X, ALU.add)
                t = wpool.tile([128, BL], DT, tag="tbr")
                nc.vector.scalar_tensor_tensor(t[:], r[:], w63, red[:],
                                               ALU.mult, ALU.subtract)
                return t

            actp = wpool.tile([128, 2 * BL], DT, tag="actp")
            actT = []

            def finish_half(half, tp, tm):
                # anc = tp - tm  (tm carries +diff weights = -w_minus terms)
                anc = wpool.tile([128, BL], DT, tag="anc")
                nc.vector.tensor_tensor(anc[:], tp[:], tm[:], ALU.subtract)
                at = actp[:, half * BL:(half + 1) * BL]
                nc.vector.tensor_scalar(at, anc[:], nma_col[:], ma_col,
                                        ALU.max, ALU.min)
                actT.append(at)

            # FAT unit (+,0) goes first so the DVE gets busy asap
            t_p0 = fat_unit(r_p[0], wpb[0], cK[0], W63p[0])

            # ---- sin / cos of delta (range-reduced; fills DVE/ACT gaps) --
            # y = x - 2pi*k via an int32 cast (round-to-nearest on HW,
            # trunc in CoreSim); a branch-free +-2pi correction makes the
            # result [-pi, pi] under either conversion mode.
            sinT, cosT = [], []
            for half in range(2):
                ki = wpool.tile([128, BL], mybir.dt.int32, tag="sc_ki")
                nc.vector.tensor_scalar(ki[:], fd_ps[2 + half][:],
                                        float(1 / (2 * PI)), 32.0, ALU.mult, ALU.add)
                xoff = wpool.tile([128, BL], DT, tag="sc_xoff")
                nc.vector.tensor_scalar(xoff[:], fd_ps[2 + half][:], float(64 * PI),
                                        None, ALU.add)
                y1 = wpool.tile([128, BL], DT, tag="sc_y1")
                nc.vector.scalar_tensor_tensor(y1[:], ki[:], float(-2 * PI), xoff[:],
                                               ALU.mult, ALU.add)
                w = wpool.tile([128, BL], DT, tag="sc_w")
                nc.vector.tensor_scalar(w[:], y1[:], PI, float(-2 * PI),
                                        ALU.is_gt, ALU.mult)
                y2 = wpool.tile([128, BL], DT, tag="sc_y2")
                nc.vector.tensor_tensor(y2[:], y1[:], w[:], ALU.add)
                y = wpool.tile([128, BL], DT, tag="sc_y")
                nc.vector.tensor_scalar(y[:], y2[:], -PI, PI, ALU.max, ALU.min)
                s = wpool.tile([128, BL], DT, tag=f"sinT{half}")
                nc.scalar.activation(s[:], y[:], ACTF.Sin)
                # cos(y) = sin(pi/2 - |y|),  argument stays in [-pi/2, pi/2]
                ay = wpool.tile([128, BL], DT, tag="sc_ay")
                nc.scalar.activation(ay[:], y[:], ACTF.Abs)
                c = wpool.tile([128, BL], DT, tag=f"cosT{half}")
                nc.scalar.activation(c[:], ay[:], ACTF.Sin, bias=bias_hpi[:],
                                     scale=-1.0)
                sinT.append(s)
                cosT.append(c)

            t_m0 = fat_unit(r_m[0], wmb[0], cKm[0], W63m[0])
            finish_half(0, t_p0, t_m0)

            # ---- U = F @ cos, V = F @ sin ; dtT = sin*U - cos*V ----------
            dtT = []
            for m in range(2):
                psU = ppool.tile([128, BL], DT, tag="ps")
                psV = ppool.tile([128, BL], DT, tag="ps")
                for k in range(2):
                    lhs = stFTp[:, k * A + m * 128:k * A + (m + 1) * 128]
                    nc.tensor.matmul(psU[:], lhs, cosT[k][:], start=(k == 0), stop=(k == 1))
                    uv_last = nc.tensor.matmul(psV[:], lhs, sinT[k][:],
                                               start=(k == 0), stop=(k == 1))
                t1 = wpool.tile([128, BL], DT, tag="dt_t1")
                nc.vector.tensor_tensor(t1[:], sinT[m][:], psU[:], ALU.mult)
                t2 = wpool.tile([128, BL], DT, tag="dt_t2")
                nc.vector.tensor_tensor(t2[:], cosT[m][:], psV[:], ALU.mult)
                t = wpool.tile([128, BL], DT, tag=f"dtT{m}")
                nc.vector.tensor_tensor(t[:], t1[:], t2[:], ALU.subtract)
                dtT.append(t)

            # ---- new_state stacked matmul: everything that doesn't need
            # action accumulates into PSUM while the basis runs -------------
            ns_ps = ppool_ns.tile([BL, S], DT, tag="ns_ps")
            ns_first = nc.tensor.matmul(ns_ps[:], ones_row[:], st3[:],
                                        start=True, stop=False)
            tile.add_dep_helper(ns_first.ins, uv_last.ins, sync=False,
                                reason="keep PE free for U/V before the wns stack")
            for k in range(4):
                nc.tensor.matmul(ns_ps[:], prevT(k), wns(k), start=False, stop=False)
            for k in range(2):
                nc.tensor.matmul(ns_ps[:], inpp[:, k * BL:(k + 1) * BL], wns(6 + k),
                                 start=False, stop=False)
            for k in range(2):
                nc.tensor.matmul(ns_ps[:], dtT[k][:], wns(4 + k),
                                 start=False, stop=False)
            # action half 0 as soon as it exists (wns block 8 = st4 rows 0:128)
            nc.tensor.matmul(ns_ps[:], actT[0], wns(8), start=False, stop=False)

            # remaining fat units
            t_p1 = fat_unit(r_p[1], wpb[1], cK[1], W63p[1])
            t_m1 = fat_unit(r_m[1], wmb[1], cKm[1], W63m[1])
            finish_half(1, t_p1, t_m1)
            nc.sync.dma_start(out=d_actp[:], in_=actp[:])
            nc.tensor.matmul(ns_ps[:], actT[1], wns(9), start=False, stop=True)

            ns_nat = wpool.tile([BL, S], DT, tag="ns_nat")
            nc.vector.tensor_copy(ns_nat[:], ns_ps[:])
            nc.sync.dma_start(out=d_ns[:], in_=ns_nat[:])

            # ---- transpose new_state back to [s, b] for the out matmuls --
            nsT, ns2T = [], []
            for k in range(4):
                ps = ppool.tile([128, BL], DT, tag="ps")
                nc.tensor.transpose(ps[:], ns_nat[:, k * 128:(k + 1) * 128], ident[:])
                t = wpool.tile([128, BL], DT, tag=f"nsT{k}")
                nc.vector.tensor_copy(t[:], ps[:])
                nsT.append(t)
                t2 = wpool.tile([128, BL], DT, tag=f"ns2T{k}")
                nc.vector.tensor_tensor(t2[:], t[:], t[:], ALU.mult)
                ns2T.append(t2)

            # ---- frequency = ns @ select_w ; loss0 = ns^2 @ saw ----------
            fq_ps = ppool.tile([BL, A], DT, tag="ps")
            for k in range(4):
                nc.tensor.matmul(fq_ps[:], nsT[k][:], selwk(k),
                                 start=(k == 0), stop=(k == 3))
            fq = wpool.tile([BL, A], DT, tag="fq")
            nc.vector.tensor_copy(fq[:], fq_ps[:])
            nc.sync.dma_start(out=d_freq[:], in_=fq[:])

            ls_ps = ppool.tile([BL, 1], DT, tag="ps")
            for k in range(4):
                nc.tensor.matmul(ls_ps[:], ns2T[k][:], sawma[:, k:k + 1],
                                 start=(k == 0), stop=(k == 3))
            ls = wpool.tile([BL, 1], DT, tag="ls")
            nc.vector.tensor_copy(ls[:], ls_ps[:])
            nc.sync.dma_start(out=d_loss[:], in_=ls[:])

    nc.compile()
    return nc


_NC_CACHE = None


def _get_nc():
    global _NC_CACHE
    if _NC_CACHE is None:
        _NC_CACHE = build_nc()
    return _NC_CACHE


def _pack_rows(a, width):
    """[R, C] with R = 128*n  ->  [128, n*C] panel (blocks along free dim)."""
    r, c = a.shape
    n = r // 128
    return np.ascontiguousarray(
        a.reshape(n, 128, c).transpose(1, 0, 2).reshape(128, n * c))


def prepare_in_maps(inputs):
    x = {k: np.asarray(v) for k, v in inputs.items()}
    f32 = lambda a: np.ascontiguousarray(a, dtype=np.float32)

    wns = np.concatenate([x["state_transfer1"], x["state_transfer2"],
                          x["state_transfer3_Pm"], x["state_transfer4"]], axis=0)
    sawma = np.zeros((128, 5), np.float32)
    sawma[:, 0:4] = x["select_add_w"].reshape(4, 128).T
    sawma[:, 4] = np.float32(x["max_action"])

    shared = {
        "pk": f32(_pack_rows(np.concatenate(
            [x["w_plus_temp0"], x["b_plus_temp0"],
             x["w_minus_temp0"], x["b_minus_temp0"]], axis=1), 4 * I)),
        "selwp": f32(_pack_rows(x["select_w"], A)),
        "seldp": f32(_pack_rows(x["select_delta"], A)),
        "sawma": sawma,
        "stFTp": f32(_pack_rows(x["state_transferF"].T, A)),
        "st3": f32(x["state_transfer3"][None, :]),
        "wnsa": f32(_pack_rows(wns[:640], S)),
        "wnsb": f32(_pack_rows(wns[640:], S)),
    }
    in_maps = []
    for k in range(NCORES):
        sl = slice(k * BL, (k + 1) * BL)
        m = dict(shared)
        m["prevp"] = f32(_pack_rows(x["prev_output"][sl].T, BL))
        m["inpp"] = f32(_pack_rows(x["inputs"][sl].T, BL))
        in_maps.append(m)
    return in_maps


def postprocess(res):
    loss0 = np.concatenate([res[k]["loss_out"] for k in range(NCORES)], axis=0)
    frequency = np.concatenate([res[k]["freq_out"] for k in range(NCORES)], axis=0)
    action = np.concatenate(
        [np.concatenate([res[k]["actp_out"][:, 0:BL],
                         res[k]["actp_out"][:, BL:2 * BL]], axis=0).T
         for k in range(NCORES)], axis=0)
    new_state = np.concatenate([res[k]["ns_out"] for k in range(NCORES)], axis=0)
    return (loss0, frequency, action, new_state)


def kernel(**inputs):
    in_maps = prepare_in_maps(inputs)
    res = run_bass_kernel_spmd(_get_nc(), in_maps, list(range(NCORES))).results
    return postprocess(res)
